# revision 3
# baseline (speedup 1.0000x reference)
"""Trainium2 Bass kernel for nn_BaseHashCode (prefix-hash of ragged sequences).

Reference computation (per row of `sequences` [B, 64], int32 digits 0..7):
    A_t  = b + sum_{i<=t} a_i * x_i                    (int, < 2^29)
    pid_t = (A_t % 1000003) % 65536   with jax-CPU int32 `%` semantics:
            accf = RNE_f32(A); t = accf - 500001.0 (f32); d = t / p (f32);
            q = round_half_away(d); r = A - q*p (int32); pid = r & 0xffff
    len   = #nonzero digits in the row
    out_t = pid_t if t < len else pid_{len-1}   (len==0 -> pid_0; all equal)

Pure data parallel over 8 NeuronCores (batch shard).  Per core, batch-major
tiles [128 partitions x 1024 free] (16 rows of 64 per partition).

Engine split (v2): the scalar/ACT engine takes every affine/convert unary op,
gpsimd only the two exponent-mask bitwise ANDs, and the vector engine the
tensor-tensor/scan/reduce work.  The modulus is exact in fp32 pieces via a
12/12 split of a (ahi = a>>11 < 512, alo = a&0x7ff < 2048), which keeps a
single continuous 1024-long prefix scan exact (< 2^24) per tile; per-64-block
carries are subtracted afterwards.  q is corrected to the oracle's
round-half-away(f32-division) with ulp-exact thresholds (see derivation in
comments below).
"""

import json

import numpy as np

import concourse.bass as bass
import concourse.mybir as mybir
from concourse.tile import TileContext
from concourse.bass_utils import run_bass_kernel_spmd


# ---------------------------------------------------------------------------
# BIR fixup: this container's walrus rejects instructions with too many
# sync_info.on_wait entries ("Too many sync wait commands").  Hoist excess
# waits onto injected same-engine NoOp instructions placed just before the
# offending instruction (same engine stream => identical semantics).  Only
# monotone waits (sem-ge-imm) are hoisted; eq-style waits stay put.
# ---------------------------------------------------------------------------
_WAIT_LIMIT = 1


def _fix_bir_sync_waits(bir_bytes: bytes, limit: int = _WAIT_LIMIT) -> bytes:
    bir = json.loads(bir_bytes)
    n_fixed = [0]

    def fix_list(insts):
        out = []
        for inst in insts:
            si = inst.get("sync_info") or {}
            ow = si.get("on_wait") or []
            if len(ow) > limit:
                movable = [w for w in ow if w.get("wait_mode") == "sem-ge-imm"]
                fixed = [w for w in ow if w.get("wait_mode") != "sem-ge-imm"]
                keep = (fixed + movable)[:limit]
                hoist = (fixed + movable)[limit:]
                if any(w.get("wait_mode") != "sem-ge-imm" for w in hoist):
                    out.append(inst)
                    continue
                for k in range(0, len(hoist), limit):
                    chunk = hoist[k : k + limit]
                    n_fixed[0] += 1
                    out.append(
                        {
                            "debug": inst.get("debug", 0),
                            "engine": inst["engine"],
                            "ins": [],
                            "name": f"{inst['name']}-wf{k}",
                            "opcode": "NoOp",
                            "outs": [],
                            "sync_info": {"on_wait": chunk},
                        }
                    )
                si = dict(si)
                si["on_wait"] = keep
                inst = dict(inst)
                inst["sync_info"] = si
            out.append(inst)
        return out

    def walk(o):
        if isinstance(o, dict):
            for k, v in o.items():
                if k == "instructions" and isinstance(v, list):
                    o[k] = fix_list(v)
                else:
                    walk(v)
        elif isinstance(o, list):
            for v in o:
                walk(v)

    walk(bir)
    if n_fixed[0]:
        return json.dumps(bir).encode()
    return bir_bytes


def _install_compile_patch():
    import concourse.bass_utils as bu
    import concourse.bass2jax as b2j

    if getattr(bu.compile_bir_kernel, "_waitfix", False):
        return
    orig = bu.compile_bir_kernel

    def patched(bir_json, tmpdir, neff_name="file.neff"):
        return orig(_fix_bir_sync_waits(bir_json), tmpdir, neff_name=neff_name)

    patched._waitfix = True
    bu.compile_bir_kernel = patched
    b2j.compile_bir_kernel = patched


_install_compile_patch()


PRIME = 1_000_003
SHIFT = 11
SCALE = 1 << SHIFT            # 2048
P_HI = PRIME >> SHIFT         # 488
P_LO = PRIME & (SCALE - 1)    # 579   (488*2048 + 579 == 1000003)
P_HI_SC = float(P_HI * SCALE) # 999424.0
L = 64
N_CORES = 8
B_TOTAL = 1_048_576
ROWS_PER_CORE = B_TOTAL // N_CORES  # 131072

FD = 1024                    # free-dim elements per tile
RB = FD // L                 # rows per partition per tile
TILE_ROWS = 128 * RB
N_TILES = ROWS_PER_CORE // TILE_ROWS

AOT = mybir.AluOpType
F32 = mybir.dt.float32
I32 = mybir.dt.int32
COPY = mybir.ActivationFunctionType.Copy
SIGN = mybir.ActivationFunctionType.Sign
RELU = mybir.ActivationFunctionType.Relu

C1 = float(np.float32(1.0) / np.float32(PRIME))
C3 = float(np.float32(PRIME / (1 << 23)))


def build_nc(b_val: int, rows: int = ROWS_PER_CORE, fd: int = FD):
    rb = fd // L
    tile_rows = 128 * rb
    n_tiles = rows // tile_rows
    assert rows % tile_rows == 0
    b_hi = float(int(b_val) >> SHIFT)
    b_lo = float(int(b_val) & (SCALE - 1))

    nc = bass.Bass(target_bir_lowering=False)
    seq = nc.declare_dram_parameter("sequences", [rows, L], I32, isOutput=False)
    ahi_rep = nc.declare_dram_parameter("ahi_rep", [128, fd], F32, isOutput=False)
    alo_rep = nc.declare_dram_parameter("alo_rep", [128, fd], F32, isOutput=False)
    io0_rep = nc.declare_dram_parameter("io0_rep", [128, fd], F32, isOutput=False)
    out = nc.declare_dram_parameter("out", [rows, L], I32, isOutput=True)

    seq_t = seq.rearrange("(n p r) l -> n p (r l)", p=128, r=rb)
    out_t = out.rearrange("(n p r) l -> n p (r l)", p=128, r=rb)

    with TileContext(nc) as tc:
        with (
            tc.tile_pool(name="consts", bufs=1) as cpool,
            tc.tile_pool(name="work", bufs=3) as wpool,
            tc.tile_pool(name="mid", bufs=1) as mpool,
        ):
            ahi_sb = cpool.tile([128, fd], F32, tag="ahi")
            alo_sb = cpool.tile([128, fd], F32, tag="alo")
            io_sb = cpool.tile([128, fd], F32, tag="io")
            nc.sync.dma_start(out=ahi_sb[:, :], in_=ahi_rep[:, :])
            nc.sync.dma_start(out=alo_sb[:, :], in_=alo_rep[:, :])
            nc.sync.dma_start(out=io_sb[:, :], in_=io0_rep[:, :])
            io3 = io_sb[:, :].rearrange("p (r l) -> p r l", l=L)

            neg1 = cpool.tile([128, 1], F32, tag="neg1")
            nc.gpsimd.memset(neg1[:, :], -1.0)
            # per-64-block carry tiles; column 0 is the zero carry of block 0
            Bhi = cpool.tile([128, rb], F32, tag="Bhi")
            Blo = cpool.tile([128, rb], F32, tag="Blo")
            nc.gpsimd.memset(Bhi[:, 0:1], 0.0)
            nc.gpsimd.memset(Blo[:, 0:1], 0.0)
            Bhi3 = Bhi[:, :].rearrange("p (r o) -> p r o", o=1)
            Blo3 = Blo[:, :].rearrange("p (r o) -> p r o", o=1)

            for n in range(n_tiles):
                x_i = wpool.tile([128, fd], I32, tag="x")
                nc.sync.dma_start(out=x_i[:, :], in_=seq_t[n])

                thi = mpool.tile([128, fd], F32, tag="thi")
                nc.vector.tensor_tensor(thi[:, :], x_i[:, :], ahi_sb[:, :], AOT.mult)
                tlo = mpool.tile([128, fd], F32, tag="tlo")
                nc.vector.tensor_tensor(tlo[:, :], x_i[:, :], alo_sb[:, :], AOT.mult)

                # continuous prefix sums over the whole 1024-wide tile (exact:
                # chi <= 16*64*7*511 + 6 < 2^24, clo <= 16*64*7*2047 + 57 < 2^24)
                chi = mpool.tile([128, fd], F32, tag="chi")
                nc.vector.tensor_tensor_scan(
                    chi[:, :], thi[:, :], thi[:, :], b_hi, AOT.add, AOT.bypass
                )
                clo = mpool.tile([128, fd], F32, tag="clo")
                nc.vector.tensor_tensor_scan(
                    clo[:, :], tlo[:, :], tlo[:, :], b_lo, AOT.add, AOT.bypass
                )
                chi3 = chi[:, :].rearrange("p (r l) -> p r l", l=L)
                clo3 = clo[:, :].rearrange("p (r l) -> p r l", l=L)

                # carry into block r = C[r*64-1] - b_part  (0 for block 0)
                nc.scalar.activation(
                    Bhi3[:, 1:rb, :], chi3[:, 0 : rb - 1, L - 1 : L], COPY, bias=-b_hi
                )
                nc.scalar.activation(
                    Blo3[:, 1:rb, :], clo3[:, 0 : rb - 1, L - 1 : L], COPY, bias=-b_lo
                )
                # ragged-tail: w = (x != 0) = Sign(xf) for x >= 0 (emitted early
                # so the DVE can reduce it while ACT handles the q-chain)
                w = mpool.tile([128, fd], F32, tag="w")
                nc.scalar.activation(w[:, :], x_i[:, :], SIGN)

                shi = mpool.tile([128, fd], F32, tag="shi")
                shi3 = shi[:, :].rearrange("p (r l) -> p r l", l=L)
                nc.vector.tensor_tensor(
                    shi3, chi3, Bhi3.broadcast_to([128, rb, L]), AOT.subtract
                )
                slo = mpool.tile([128, fd], F32, tag="slo")
                slo3 = slo[:, :].rearrange("p (r l) -> p r l", l=L)
                nc.vector.tensor_tensor(
                    slo3, clo3, Blo3.broadcast_to([128, rb, L]), AOT.subtract
                )

                # A = 2048*shi + slo (exact int);  accf = RNE(A) matches oracle
                accf = mpool.tile([128, fd], F32, tag="accf")
                nc.vector.scalar_tensor_tensor(
                    accf[:, :], shi[:, :], float(SCALE), slo[:, :], AOT.mult, AOT.add
                )
                t = mpool.tile([128, fd], F32, tag="t")
                nc.scalar.activation(t[:, :], accf[:, :], COPY, bias=-500001.0)

                # q0 = rne(t*c1); oracle wants q = half_away(RNE_f32(t/p)).
                # |q0 - t/p| < 0.51, so q in {q0-1, q0, q0+1}; the up/down
                # corrections below test t/p against the f32 rounding
                # thresholds of the division: up iff p*ulp(q0+0.5) >= G :=
                # p - 2*(t - q0*p), down iff p*ulp(q0-0.5) < Gm := G - 2p
                # (exact; boundaries are never hit: rho integer, p odd).
                q0 = mpool.tile([128, fd], I32, tag="q0")
                nc.scalar.activation(q0[:, :], t[:, :], COPY, scale=C1)
                qp5 = mpool.tile([128, fd], F32, tag="qp5")
                nc.scalar.activation(qp5[:, :], q0[:, :], COPY, bias=0.5)
                qm5 = mpool.tile([128, fd], F32, tag="qm5")
                nc.scalar.activation(qm5[:, :], q0[:, :], COPY, bias=-0.5)
                qhp = mpool.tile([128, fd], F32, tag="qhp")
                nc.scalar.activation(qhp[:, :], q0[:, :], COPY, scale=P_HI_SC)

                # DVE fills the ACT round-trip with the ragged-tail reduce
                lens = mpool.tile([128, rb, 1], F32, tag="lens")
                nc.vector.tensor_reduce(
                    lens[:, :, :],
                    w[:, :].rearrange("p (r l) -> p r l", l=L),
                    mybir.AxisListType.X,
                    AOT.add,
                )
                nmask = mpool.tile([128, fd], mybir.dt.uint32, tag="nmask")
                nmask3 = nmask[:, :].rearrange("p (r l) -> p r l", l=L)
                nc.vector.tensor_tensor(
                    nmask3, io3, lens[:, :, :].broadcast_to([128, rb, L]), AOT.is_ge
                )
                lm1 = mpool.tile([128, rb, 1], F32, tag="lm1")
                nc.scalar.activation(
                    lm1[:, :, :], lens[:, :, :], RELU, bias=neg1[:, :]
                )

                s1 = mpool.tile([128, fd], F32, tag="s1")
                nc.vector.tensor_tensor(s1[:, :], t[:, :], qhp[:, :], AOT.subtract)
                rxd = mpool.tile([128, fd], F32, tag="rxd")
                nc.vector.scalar_tensor_tensor(
                    rxd[:, :], q0[:, :], -float(P_LO), s1[:, :], AOT.mult, AOT.add
                )
                G = mpool.tile([128, fd], F32, tag="G")
                nc.scalar.activation(
                    G[:, :], rxd[:, :], COPY, scale=-2.0, bias=float(PRIME)
                )
                Gm = mpool.tile([128, fd], F32, tag="Gm")
                nc.scalar.activation(
                    Gm[:, :], rxd[:, :], COPY, scale=-2.0, bias=-float(PRIME)
                )
                ebu = mpool.tile([128, fd], I32, tag="ebu")
                nc.vector.tensor_scalar(
                    ebu[:, :], qp5[:, :].bitcast(I32), 0x7F800000, None, AOT.bitwise_and
                )
                ebd = mpool.tile([128, fd], I32, tag="ebd")
                nc.vector.tensor_scalar(
                    ebd[:, :], qm5[:, :].bitcast(I32), 0x7F800000, None, AOT.bitwise_and
                )
                Vu = mpool.tile([128, fd], F32, tag="Vu")
                nc.scalar.activation(Vu[:, :], ebu[:, :].bitcast(F32), COPY, scale=C3)
                Vd = mpool.tile([128, fd], F32, tag="Vd")
                nc.scalar.activation(Vd[:, :], ebd[:, :].bitcast(F32), COPY, scale=C3)

                oh = mpool.tile([128, fd], I32, tag="oh")
                oh3 = oh[:, :].rearrange("p (r l) -> p r l", l=L)
                nc.vector.tensor_tensor(
                    oh3, io3.bitcast(I32),
                    lm1[:, :, :].bitcast(I32).broadcast_to([128, rb, L]), AOT.is_equal
                )

                up = mpool.tile([128, fd], I32, tag="up")
                nc.vector.tensor_tensor(
                    up[:, :], Vu[:, :].bitcast(I32), G[:, :].bitcast(I32), AOT.is_ge
                )
                down = mpool.tile([128, fd], I32, tag="down")
                nc.vector.tensor_tensor(
                    down[:, :], Gm[:, :].bitcast(I32), Vd[:, :].bitcast(I32), AOT.is_ge
                )
                du = mpool.tile([128, fd], I32, tag="du")
                nc.vector.tensor_tensor(du[:, :], up[:, :], down[:, :], AOT.subtract)

                # exact residual r = A - q*p from the exact pieces
                u2 = mpool.tile([128, fd], F32, tag="u2")
                nc.vector.scalar_tensor_tensor(
                    u2[:, :], q0[:, :], -float(P_HI), shi[:, :], AOT.mult, AOT.add
                )
                v2 = mpool.tile([128, fd], F32, tag="v2")
                nc.vector.scalar_tensor_tensor(
                    v2[:, :], q0[:, :], -float(P_LO), slo[:, :], AOT.mult, AOT.add
                )
                r0 = mpool.tile([128, fd], F32, tag="r0")
                nc.vector.scalar_tensor_tensor(
                    r0[:, :], u2[:, :], float(SCALE), v2[:, :], AOT.mult, AOT.add
                )
                rref = mpool.tile([128, fd], I32, tag="rref")
                nc.vector.scalar_tensor_tensor(
                    rref[:, :], du[:, :], -float(PRIME), r0[:, :], AOT.mult, AOT.add
                )
                pid = mpool.tile([128, fd], I32, tag="pid")
                nc.vector.tensor_scalar(
                    pid[:, :], rref[:, :], 65535, None, AOT.bitwise_and
                )
                pidf = mpool.tile([128, fd], F32, tag="pidf")
                nc.scalar.activation(pidf[:, :], pid[:, :], COPY)
                pidf3 = pidf[:, :].rearrange("p (r l) -> p r l", l=L)

                ohp = mpool.tile([128, fd], F32, tag="ohp")
                nc.vector.tensor_tensor(ohp[:, :], oh[:, :], pidf[:, :], AOT.mult)
                C = mpool.tile([128, rb, 1], F32, tag="C")
                nc.vector.tensor_reduce(
                    C[:, :, :],
                    ohp[:, :].rearrange("p (r l) -> p r l", l=L),
                    mybir.AxisListType.X,
                    AOT.add,
                )
                C3b = C[:, :, :].broadcast_to([128, rb, L])

                # positions t >= len get C = pid[len-1] (in-place overwrite)
                nc.vector.copy_predicated(pidf3, nmask3, C3b)

                o = wpool.tile([128, fd], I32, tag="o")
                nc.scalar.activation(o[:, :], pidf[:, :], COPY)

                nc.sync.dma_start(out=out_t[n], in_=o[:, :])

    return nc


_NC_CACHE: dict = {}


def _get_nc(b_val: int):
    key = (int(b_val), ROWS_PER_CORE, FD)
    if key not in _NC_CACHE:
        _NC_CACHE[key] = build_nc(int(b_val))
    return _NC_CACHE[key]


def make_const_inputs(a: np.ndarray, fd: int = FD):
    rb = fd // L
    a64 = a.astype(np.int64)
    ahi_rep = np.tile((a64 >> SHIFT).astype(np.float32), (128, rb))
    alo_rep = np.tile((a64 & (SCALE - 1)).astype(np.float32), (128, rb))
    io0_rep = np.tile(np.arange(L, dtype=np.float32), (128, rb))
    return ahi_rep, alo_rep, io0_rep


def make_in_maps(sequences: np.ndarray, a: np.ndarray):
    ahi_rep, alo_rep, io0_rep = make_const_inputs(a)
    in_maps = []
    for i in range(N_CORES):
        shard = np.ascontiguousarray(
            sequences[i * ROWS_PER_CORE : (i + 1) * ROWS_PER_CORE].astype(
                np.int32, copy=False
            )
        )
        in_maps.append(
            {
                "sequences": shard,
                "ahi_rep": ahi_rep,
                "alo_rep": alo_rep,
                "io0_rep": io0_rep,
            }
        )
    return in_maps


def kernel(sequences: np.ndarray, a: np.ndarray, b) -> np.ndarray:
    sequences = np.asarray(sequences)
    a = np.asarray(a)
    assert sequences.shape == (B_TOTAL, L), sequences.shape

    nc = _get_nc(int(b))
    in_maps = make_in_maps(sequences, a)
    res = run_bass_kernel_spmd(nc, in_maps, core_ids=list(range(N_CORES)))
    outs = [res.results[i]["out"] for i in range(N_CORES)]
    return np.concatenate(outs, axis=0).astype(np.int32, copy=False)


if __name__ == "__main__":
    rng = np.random.default_rng(0)
    seqs = rng.integers(0, 8, size=(B_TOTAL, L), dtype=np.int32)
    a = rng.integers(1, PRIME, size=(L,), dtype=np.int32)
    out = kernel(sequences=seqs, a=a, b=12345)
    print(out.shape, out.dtype, out[:2, :8])


# revision 4
# speedup vs baseline: 1.0331x; 1.0331x over previous
"""Trainium2 Bass kernel for nn_BaseHashCode (prefix-hash of ragged sequences).

Reference computation (per row of `sequences` [B, 64], int32 digits 0..7):
    A_t  = b + sum_{i<=t} a_i * x_i                    (int, < 2^29)
    pid_t = (A_t % 1000003) % 65536   with jax-CPU int32 `%` semantics:
            accf = RNE_f32(A); t = accf - 500001.0 (f32); d = t / p (f32);
            q = round_half_away(d); r = A - q*p (int32); pid = r & 0xffff
    len   = #nonzero digits in the row
    out_t = pid_t if t < len else pid_{len-1}   (len==0 -> pid_0; all equal)

Pure data parallel over 8 NeuronCores (batch shard).  Per core, batch-major
tiles [128 partitions x 1024 free] (16 rows of 64 per partition).

Engine split (v2): the scalar/ACT engine takes every affine/convert unary op,
gpsimd only the two exponent-mask bitwise ANDs, and the vector engine the
tensor-tensor/scan/reduce work.  The modulus is exact in fp32 pieces via a
12/12 split of a (ahi = a>>11 < 512, alo = a&0x7ff < 2048), which keeps a
single continuous 1024-long prefix scan exact (< 2^24) per tile; per-64-block
carries are subtracted afterwards.  q is corrected to the oracle's
round-half-away(f32-division) with ulp-exact thresholds (see derivation in
comments below).
"""

import json

import numpy as np

import concourse.bass as bass
import concourse.mybir as mybir
from concourse.tile import TileContext
from concourse.bass_utils import run_bass_kernel_spmd


# ---------------------------------------------------------------------------
# BIR fixup: this container's walrus rejects instructions with too many
# sync_info.on_wait entries ("Too many sync wait commands").  Hoist excess
# waits onto injected same-engine NoOp instructions placed just before the
# offending instruction (same engine stream => identical semantics).  Only
# monotone waits (sem-ge-imm) are hoisted; eq-style waits stay put.
# ---------------------------------------------------------------------------
_WAIT_LIMIT = 1


def _fix_bir_sync_waits(bir_bytes: bytes, limit: int = _WAIT_LIMIT) -> bytes:
    bir = json.loads(bir_bytes)
    n_fixed = [0]

    def fix_list(insts):
        out = []
        for inst in insts:
            si = inst.get("sync_info") or {}
            ow = si.get("on_wait") or []
            if len(ow) > limit:
                movable = [w for w in ow if w.get("wait_mode") == "sem-ge-imm"]
                fixed = [w for w in ow if w.get("wait_mode") != "sem-ge-imm"]
                keep = (fixed + movable)[:limit]
                hoist = (fixed + movable)[limit:]
                if any(w.get("wait_mode") != "sem-ge-imm" for w in hoist):
                    out.append(inst)
                    continue
                for k in range(0, len(hoist), limit):
                    chunk = hoist[k : k + limit]
                    n_fixed[0] += 1
                    out.append(
                        {
                            "debug": inst.get("debug", 0),
                            "engine": inst["engine"],
                            "ins": [],
                            "name": f"{inst['name']}-wf{k}",
                            "opcode": "NoOp",
                            "outs": [],
                            "sync_info": {"on_wait": chunk},
                        }
                    )
                si = dict(si)
                si["on_wait"] = keep
                inst = dict(inst)
                inst["sync_info"] = si
            out.append(inst)
        return out

    def walk(o):
        if isinstance(o, dict):
            for k, v in o.items():
                if k == "instructions" and isinstance(v, list):
                    o[k] = fix_list(v)
                else:
                    walk(v)
        elif isinstance(o, list):
            for v in o:
                walk(v)

    walk(bir)
    if n_fixed[0]:
        return json.dumps(bir).encode()
    return bir_bytes


def _install_compile_patch():
    import concourse.bass_utils as bu
    import concourse.bass2jax as b2j

    if getattr(bu.compile_bir_kernel, "_waitfix", False):
        return
    orig = bu.compile_bir_kernel

    def patched(bir_json, tmpdir, neff_name="file.neff"):
        return orig(_fix_bir_sync_waits(bir_json), tmpdir, neff_name=neff_name)

    patched._waitfix = True
    bu.compile_bir_kernel = patched
    b2j.compile_bir_kernel = patched


_install_compile_patch()


PRIME = 1_000_003
SHIFT = 10
SCALE = 1 << SHIFT            # 1024
P_HI = PRIME >> SHIFT         # 976
P_LO = PRIME & (SCALE - 1)    # 579   (976*1024 + 579 == 1000003)
P_HI_SC = float(P_HI * SCALE) # 999424.0
L = 64
N_CORES = 8
B_TOTAL = 1_048_576
ROWS_PER_CORE = B_TOTAL // N_CORES  # 131072

FD = 2048                    # free-dim elements per tile
RB = FD // L                 # rows per partition per tile
TILE_ROWS = 128 * RB
N_TILES = ROWS_PER_CORE // TILE_ROWS

AOT = mybir.AluOpType
F32 = mybir.dt.float32
I32 = mybir.dt.int32
COPY = mybir.ActivationFunctionType.Copy
SIGN = mybir.ActivationFunctionType.Sign
RELU = mybir.ActivationFunctionType.Relu

C1 = float(np.float32(1.0) / np.float32(PRIME))
C3 = float(np.float32(PRIME / (1 << 23)))


def build_nc(b_val: int, rows: int = ROWS_PER_CORE, fd: int = FD):
    rb = fd // L
    tile_rows = 128 * rb
    n_tiles = rows // tile_rows
    assert rows % tile_rows == 0
    b_hi = float(int(b_val) >> SHIFT)
    b_lo = float(int(b_val) & (SCALE - 1))

    nc = bass.Bass(target_bir_lowering=False)
    seq = nc.declare_dram_parameter("sequences", [rows, L], I32, isOutput=False)
    ahi_rep = nc.declare_dram_parameter("ahi_rep", [128, fd], F32, isOutput=False)
    alo_rep = nc.declare_dram_parameter("alo_rep", [128, fd], F32, isOutput=False)
    io0_rep = nc.declare_dram_parameter("io0_rep", [128, fd], F32, isOutput=False)
    out = nc.declare_dram_parameter("out", [rows, L], I32, isOutput=True)

    seq_t = seq.rearrange("(n p r) l -> n p (r l)", p=128, r=rb)
    out_t = out.rearrange("(n p r) l -> n p (r l)", p=128, r=rb)

    with TileContext(nc) as tc:
        with (
            tc.tile_pool(name="consts", bufs=1) as cpool,
            tc.tile_pool(name="work", bufs=2) as wpool,
            tc.tile_pool(name="mid", bufs=1) as mpool,
        ):
            ahi_sb = cpool.tile([128, fd], F32, tag="ahi")
            alo_sb = cpool.tile([128, fd], F32, tag="alo")
            io_sb = cpool.tile([128, fd], F32, tag="io")
            nc.sync.dma_start(out=ahi_sb[:, :], in_=ahi_rep[:, :])
            nc.sync.dma_start(out=alo_sb[:, :], in_=alo_rep[:, :])
            nc.sync.dma_start(out=io_sb[:, :], in_=io0_rep[:, :])
            io3 = io_sb[:, :].rearrange("p (r l) -> p r l", l=L)

            neg1 = cpool.tile([128, 1], F32, tag="neg1")
            nc.gpsimd.memset(neg1[:, :], -1.0)
            # per-64-block carry tiles; column 0 is the zero carry of block 0
            Bhi = cpool.tile([128, rb], F32, tag="Bhi")
            Blo = cpool.tile([128, rb], F32, tag="Blo")
            nc.gpsimd.memset(Bhi[:, 0:1], 0.0)
            nc.gpsimd.memset(Blo[:, 0:1], 0.0)
            Bhi3 = Bhi[:, :].rearrange("p (r o) -> p r o", o=1)
            Blo3 = Blo[:, :].rearrange("p (r o) -> p r o", o=1)

            for n in range(n_tiles):
                x_i = wpool.tile([128, fd], I32, tag="x")
                nc.sync.dma_start(out=x_i[:, :], in_=seq_t[n])

                thi = mpool.tile([128, fd], F32, tag="scr", bufs=12)
                nc.vector.tensor_tensor(thi[:, :], x_i[:, :], ahi_sb[:, :], AOT.mult)
                tlo = mpool.tile([128, fd], F32, tag="scr", bufs=12)
                nc.vector.tensor_tensor(tlo[:, :], x_i[:, :], alo_sb[:, :], AOT.mult)

                # continuous prefix sums over the whole 2048-wide tile (exact:
                # chi <= 32*64*7*1023 + 12 < 2^24, clo <= 32*64*7*1023 + 57 < 2^24)
                chi = mpool.tile([128, fd], F32, tag="scr", bufs=12)
                nc.vector.tensor_tensor_scan(
                    chi[:, :], thi[:, :], thi[:, :], b_hi, AOT.add, AOT.bypass
                )
                clo = mpool.tile([128, fd], F32, tag="scr", bufs=12)
                nc.vector.tensor_tensor_scan(
                    clo[:, :], tlo[:, :], tlo[:, :], b_lo, AOT.add, AOT.bypass
                )
                chi3 = chi[:, :].rearrange("p (r l) -> p r l", l=L)
                clo3 = clo[:, :].rearrange("p (r l) -> p r l", l=L)

                nc.scalar.activation(
                    Bhi3[:, 1:rb, :], chi3[:, 0 : rb - 1, L - 1 : L], COPY, bias=-b_hi
                )
                nc.scalar.activation(
                    Blo3[:, 1:rb, :], clo3[:, 0 : rb - 1, L - 1 : L], COPY, bias=-b_lo
                )
                w = mpool.tile([128, fd], F32, tag="scr", bufs=12)
                nc.scalar.activation(w[:, :], x_i[:, :], SIGN)

                shi = mpool.tile([128, fd], F32, tag="shi")
                shi3 = shi[:, :].rearrange("p (r l) -> p r l", l=L)
                nc.vector.tensor_tensor(
                    shi3, chi3, Bhi3.broadcast_to([128, rb, L]), AOT.subtract
                )
                slo = mpool.tile([128, fd], F32, tag="slo")
                slo3 = slo[:, :].rearrange("p (r l) -> p r l", l=L)
                nc.vector.tensor_tensor(
                    slo3, clo3, Blo3.broadcast_to([128, rb, L]), AOT.subtract
                )

                accf = mpool.tile([128, fd], F32, tag="scr", bufs=12)
                nc.vector.scalar_tensor_tensor(
                    accf[:, :], shi[:, :], float(SCALE), slo[:, :], AOT.mult, AOT.add
                )
                t = mpool.tile([128, fd], F32, tag="scr", bufs=12)
                nc.scalar.activation(t[:, :], accf[:, :], COPY, bias=-500001.0)
                q0 = mpool.tile([128, fd], I32, tag="q0")
                nc.scalar.activation(q0[:, :], t[:, :], COPY, scale=C1)
                # qhp first: it gates the V chain (s1); qp5/qm5 are needed later
                qhp = mpool.tile([128, fd], F32, tag="scr", bufs=12)
                nc.scalar.activation(qhp[:, :], q0[:, :], COPY, scale=P_HI_SC)

                # V fills the ACT round-trip with the ragged-tail work
                lens = mpool.tile([128, rb, 1], F32, tag="lens")
                nc.vector.tensor_reduce(
                    lens[:, :, :],
                    w[:, :].rearrange("p (r l) -> p r l", l=L),
                    mybir.AxisListType.X,
                    AOT.add,
                )
                nmask = mpool.tile([128, fd], mybir.dt.uint32, tag="nmask")
                nmask3 = nmask[:, :].rearrange("p (r l) -> p r l", l=L)
                nc.vector.tensor_tensor(
                    nmask3, io3, lens[:, :, :].broadcast_to([128, rb, L]), AOT.is_ge
                )
                lm1 = mpool.tile([128, rb, 1], F32, tag="lm1")
                nc.scalar.activation(
                    lm1[:, :, :], lens[:, :, :], RELU, bias=neg1[:, :]
                )
                qp5 = mpool.tile([128, fd], F32, tag="scr", bufs=12)
                nc.scalar.activation(qp5[:, :], q0[:, :], COPY, bias=0.5)
                qm5 = mpool.tile([128, fd], F32, tag="scr", bufs=12)
                nc.scalar.activation(qm5[:, :], q0[:, :], COPY, bias=-0.5)

                s1 = mpool.tile([128, fd], F32, tag="scr", bufs=12)
                nc.vector.tensor_tensor(s1[:, :], t[:, :], qhp[:, :], AOT.subtract)
                rxd = mpool.tile([128, fd], F32, tag="scr", bufs=12)
                nc.vector.scalar_tensor_tensor(
                    rxd[:, :], q0[:, :], -float(P_LO), s1[:, :], AOT.mult, AOT.add
                )
                oh = mpool.tile([128, fd], I32, tag="oh")
                oh3 = oh[:, :].rearrange("p (r l) -> p r l", l=L)
                nc.vector.tensor_tensor(
                    oh3, io3.bitcast(I32),
                    lm1[:, :, :].bitcast(I32).broadcast_to([128, rb, L]), AOT.is_equal
                )
                G = mpool.tile([128, fd], F32, tag="scr", bufs=12)
                nc.scalar.activation(
                    G[:, :], rxd[:, :], COPY, scale=-2.0, bias=float(PRIME)
                )
                Gm = mpool.tile([128, fd], F32, tag="scr", bufs=12)
                nc.scalar.activation(
                    Gm[:, :], rxd[:, :], COPY, scale=-2.0, bias=-float(PRIME)
                )
                ebu = mpool.tile([128, fd], I32, tag="scr", bufs=12)
                nc.vector.tensor_scalar(
                    ebu[:, :], qp5[:, :].bitcast(I32), 0x7F800000, None, AOT.bitwise_and
                )
                ebd = mpool.tile([128, fd], I32, tag="scr", bufs=12)
                nc.vector.tensor_scalar(
                    ebd[:, :], qm5[:, :].bitcast(I32), 0x7F800000, None, AOT.bitwise_and
                )
                Vu = mpool.tile([128, fd], F32, tag="scr", bufs=12)
                nc.scalar.activation(Vu[:, :], ebu[:, :].bitcast(F32), COPY, scale=C3)
                Vd = mpool.tile([128, fd], F32, tag="scr", bufs=12)
                nc.scalar.activation(Vd[:, :], ebd[:, :].bitcast(F32), COPY, scale=C3)
                up = mpool.tile([128, fd], I32, tag="scr", bufs=12)
                nc.vector.tensor_tensor(
                    up[:, :], Vu[:, :].bitcast(I32), G[:, :].bitcast(I32), AOT.is_ge
                )
                down = mpool.tile([128, fd], I32, tag="scr", bufs=12)
                nc.vector.tensor_tensor(
                    down[:, :], Gm[:, :].bitcast(I32), Vd[:, :].bitcast(I32), AOT.is_ge
                )
                du = mpool.tile([128, fd], I32, tag="scr", bufs=12)
                nc.vector.tensor_tensor(du[:, :], up[:, :], down[:, :], AOT.subtract)

                u2 = mpool.tile([128, fd], F32, tag="scr", bufs=12)
                nc.vector.scalar_tensor_tensor(
                    u2[:, :], q0[:, :], -float(P_HI), shi[:, :], AOT.mult, AOT.add
                )
                v2 = mpool.tile([128, fd], F32, tag="scr", bufs=12)
                nc.vector.scalar_tensor_tensor(
                    v2[:, :], q0[:, :], -float(P_LO), slo[:, :], AOT.mult, AOT.add
                )
                r0 = mpool.tile([128, fd], F32, tag="scr", bufs=12)
                nc.vector.scalar_tensor_tensor(
                    r0[:, :], u2[:, :], float(SCALE), v2[:, :], AOT.mult, AOT.add
                )
                rref = mpool.tile([128, fd], I32, tag="scr", bufs=12)
                nc.vector.scalar_tensor_tensor(
                    rref[:, :], du[:, :], -float(PRIME), r0[:, :], AOT.mult, AOT.add
                )
                pid = mpool.tile([128, fd], I32, tag="scr", bufs=12)
                nc.vector.tensor_scalar(
                    pid[:, :], rref[:, :], 65535, None, AOT.bitwise_and
                )
                pidf = mpool.tile([128, fd], F32, tag="pidf")
                nc.scalar.activation(pidf[:, :], pid[:, :], COPY)
                pidf3 = pidf[:, :].rearrange("p (r l) -> p r l", l=L)

                ohp = mpool.tile([128, fd], F32, tag="scr", bufs=12)
                nc.vector.tensor_tensor(ohp[:, :], oh[:, :], pidf[:, :], AOT.mult)
                C = mpool.tile([128, rb, 1], F32, tag="C")
                nc.vector.tensor_reduce(
                    C[:, :, :],
                    ohp[:, :].rearrange("p (r l) -> p r l", l=L),
                    mybir.AxisListType.X,
                    AOT.add,
                )
                C3b = C[:, :, :].broadcast_to([128, rb, L])

                nc.vector.copy_predicated(pidf3, nmask3, C3b)

                o = wpool.tile([128, fd], I32, tag="o")
                nc.scalar.activation(o[:, :], pidf[:, :], COPY)

                nc.sync.dma_start(out=out_t[n], in_=o[:, :])

    return nc


_NC_CACHE: dict = {}


def _get_nc(b_val: int):
    key = (int(b_val), ROWS_PER_CORE, FD)
    if key not in _NC_CACHE:
        _NC_CACHE[key] = build_nc(int(b_val))
    return _NC_CACHE[key]


def make_const_inputs(a: np.ndarray, fd: int = FD):
    rb = fd // L
    a64 = a.astype(np.int64)
    ahi_rep = np.tile((a64 >> SHIFT).astype(np.float32), (128, rb))
    alo_rep = np.tile((a64 & (SCALE - 1)).astype(np.float32), (128, rb))
    io0_rep = np.tile(np.arange(L, dtype=np.float32), (128, rb))
    return ahi_rep, alo_rep, io0_rep


def make_in_maps(sequences: np.ndarray, a: np.ndarray):
    ahi_rep, alo_rep, io0_rep = make_const_inputs(a)
    in_maps = []
    for i in range(N_CORES):
        shard = np.ascontiguousarray(
            sequences[i * ROWS_PER_CORE : (i + 1) * ROWS_PER_CORE].astype(
                np.int32, copy=False
            )
        )
        in_maps.append(
            {
                "sequences": shard,
                "ahi_rep": ahi_rep,
                "alo_rep": alo_rep,
                "io0_rep": io0_rep,
            }
        )
    return in_maps


def kernel(sequences: np.ndarray, a: np.ndarray, b) -> np.ndarray:
    sequences = np.asarray(sequences)
    a = np.asarray(a)
    assert sequences.shape == (B_TOTAL, L), sequences.shape

    nc = _get_nc(int(b))
    in_maps = make_in_maps(sequences, a)
    res = run_bass_kernel_spmd(nc, in_maps, core_ids=list(range(N_CORES)))
    outs = [res.results[i]["out"] for i in range(N_CORES)]
    return np.concatenate(outs, axis=0).astype(np.int32, copy=False)


if __name__ == "__main__":
    rng = np.random.default_rng(0)
    seqs = rng.integers(0, 8, size=(B_TOTAL, L), dtype=np.int32)
    a = rng.integers(1, PRIME, size=(L,), dtype=np.int32)
    out = kernel(sequences=seqs, a=a, b=12345)
    print(out.shape, out.dtype, out[:2, :8])


# revision 5
# speedup vs baseline: 1.1067x; 1.0712x over previous
"""Trainium2 Bass kernel for nn_BaseHashCode (prefix-hash of ragged sequences).

Reference computation (per row of `sequences` [B, 64], int32 digits 0..7):
    A_t  = b + sum_{i<=t} a_i * x_i                    (int, < 2^29)
    pid_t = (A_t % 1000003) % 65536   with jax-CPU int32 `%` semantics:
            accf = RNE_f32(A); t = accf - 500001.0 (f32); d = t / p (f32);
            q = round_half_away(d); r = A - q*p (int32); pid = r & 0xffff
    len   = #nonzero digits in the row
    out_t = pid_t if t < len else pid_{len-1}   (len==0 -> pid_0; all equal)

Pure data parallel over 8 NeuronCores (batch shard).  Per core, batch-major
tiles [128 partitions x 1024 free] (16 rows of 64 per partition).

Engine split (v2): the scalar/ACT engine takes every affine/convert unary op,
gpsimd only the two exponent-mask bitwise ANDs, and the vector engine the
tensor-tensor/scan/reduce work.  The modulus is exact in fp32 pieces via a
12/12 split of a (ahi = a>>11 < 512, alo = a&0x7ff < 2048), which keeps a
single continuous 1024-long prefix scan exact (< 2^24) per tile; per-64-block
carries are subtracted afterwards.  q is corrected to the oracle's
round-half-away(f32-division) with ulp-exact thresholds (see derivation in
comments below).
"""

import json

import numpy as np

import concourse.bass as bass
import concourse.mybir as mybir
from concourse.tile import TileContext
from concourse.bass_utils import run_bass_kernel_spmd


# ---------------------------------------------------------------------------
# BIR fixup: this container's walrus rejects instructions with too many
# sync_info.on_wait entries ("Too many sync wait commands").  Hoist excess
# waits onto injected same-engine NoOp instructions placed just before the
# offending instruction (same engine stream => identical semantics).  Only
# monotone waits (sem-ge-imm) are hoisted; eq-style waits stay put.
# ---------------------------------------------------------------------------
_WAIT_LIMIT = 1


def _fix_bir_sync_waits(bir_bytes: bytes, limit: int = _WAIT_LIMIT) -> bytes:
    bir = json.loads(bir_bytes)
    n_fixed = [0]

    def fix_list(insts):
        out = []
        for inst in insts:
            si = inst.get("sync_info") or {}
            ow = si.get("on_wait") or []
            if len(ow) > limit:
                movable = [w for w in ow if w.get("wait_mode") == "sem-ge-imm"]
                fixed = [w for w in ow if w.get("wait_mode") != "sem-ge-imm"]
                keep = (fixed + movable)[:limit]
                hoist = (fixed + movable)[limit:]
                if any(w.get("wait_mode") != "sem-ge-imm" for w in hoist):
                    out.append(inst)
                    continue
                for k in range(0, len(hoist), limit):
                    chunk = hoist[k : k + limit]
                    n_fixed[0] += 1
                    out.append(
                        {
                            "debug": inst.get("debug", 0),
                            "engine": inst["engine"],
                            "ins": [],
                            "name": f"{inst['name']}-wf{k}",
                            "opcode": "NoOp",
                            "outs": [],
                            "sync_info": {"on_wait": chunk},
                        }
                    )
                si = dict(si)
                si["on_wait"] = keep
                inst = dict(inst)
                inst["sync_info"] = si
            out.append(inst)
        return out

    def walk(o):
        if isinstance(o, dict):
            for k, v in o.items():
                if k == "instructions" and isinstance(v, list):
                    o[k] = fix_list(v)
                else:
                    walk(v)
        elif isinstance(o, list):
            for v in o:
                walk(v)

    walk(bir)
    if n_fixed[0]:
        return json.dumps(bir).encode()
    return bir_bytes


def _install_compile_patch():
    import concourse.bass_utils as bu
    import concourse.bass2jax as b2j

    if getattr(bu.compile_bir_kernel, "_waitfix", False):
        return
    orig = bu.compile_bir_kernel

    def patched(bir_json, tmpdir, neff_name="file.neff"):
        return orig(_fix_bir_sync_waits(bir_json), tmpdir, neff_name=neff_name)

    patched._waitfix = True
    bu.compile_bir_kernel = patched
    b2j.compile_bir_kernel = patched


_install_compile_patch()


PRIME = 1_000_003
SHIFT = 10
SCALE = 1 << SHIFT            # 1024
P_HI = PRIME >> SHIFT         # 976
P_LO = PRIME & (SCALE - 1)    # 579   (976*1024 + 579 == 1000003)
P_HI_SC = float(P_HI * SCALE) # 999424.0
L = 64
N_CORES = 8
B_TOTAL = 1_048_576
ROWS_PER_CORE = B_TOTAL // N_CORES  # 131072

FD = 2048                    # free-dim elements per tile
RB = FD // L                 # rows per partition per tile
TILE_ROWS = 128 * RB
N_TILES = ROWS_PER_CORE // TILE_ROWS

AOT = mybir.AluOpType
F32 = mybir.dt.float32
I32 = mybir.dt.int32
COPY = mybir.ActivationFunctionType.Copy
SIGN = mybir.ActivationFunctionType.Sign
RELU = mybir.ActivationFunctionType.Relu

C1 = float(np.float32(1.0) / np.float32(PRIME))
C3 = float(np.float32(PRIME / (1 << 23)))


def build_nc(b_val: int, rows: int = ROWS_PER_CORE, fd: int = FD):
    rb = fd // L
    tile_rows = 128 * rb
    n_tiles = rows // tile_rows
    assert rows % tile_rows == 0
    b_hi = float(int(b_val) >> SHIFT)
    b_lo = float(int(b_val) & (SCALE - 1))

    nc = bass.Bass(target_bir_lowering=False)
    seq = nc.declare_dram_parameter("sequences", [rows, L], I32, isOutput=False)
    ahi_rep = nc.declare_dram_parameter("ahi_rep", [128, fd], F32, isOutput=False)
    alo_rep = nc.declare_dram_parameter("alo_rep", [128, fd], F32, isOutput=False)
    io0_rep = nc.declare_dram_parameter("io0_rep", [128, fd], F32, isOutput=False)
    out = nc.declare_dram_parameter("out", [rows, L], I32, isOutput=True)

    seq_t = seq.rearrange("(n p r) l -> n p (r l)", p=128, r=rb)
    out_t = out.rearrange("(n p r) l -> n p (r l)", p=128, r=rb)

    with TileContext(nc) as tc:
        with (
            tc.tile_pool(name="consts", bufs=1) as cpool,
            tc.tile_pool(name="work", bufs=2) as wpool,
            tc.tile_pool(name="mid", bufs=1) as mpool,
        ):
            ahi_sb = cpool.tile([128, fd], F32, tag="ahi")
            alo_sb = cpool.tile([128, fd], F32, tag="alo")
            io_sb = cpool.tile([128, fd], F32, tag="io")
            nc.sync.dma_start(out=ahi_sb[:, :], in_=ahi_rep[:, :])
            nc.sync.dma_start(out=alo_sb[:, :], in_=alo_rep[:, :])
            nc.sync.dma_start(out=io_sb[:, :], in_=io0_rep[:, :])
            io3 = io_sb[:, :].rearrange("p (r l) -> p r l", l=L)

            neg1 = cpool.tile([128, 1], F32, tag="neg1")
            nc.gpsimd.memset(neg1[:, :], -1.0)
            # per-64-block carry tiles; column 0 is the zero carry of block 0
            Bhi = cpool.tile([128, rb], F32, tag="Bhi")
            Blo = cpool.tile([128, rb], F32, tag="Blo")
            nc.gpsimd.memset(Bhi[:, 0:1], 0.0)
            nc.gpsimd.memset(Blo[:, 0:1], 0.0)
            Bhi3 = Bhi[:, :].rearrange("p (r o) -> p r o", o=1)
            Blo3 = Blo[:, :].rearrange("p (r o) -> p r o", o=1)

            for n in range(n_tiles):
                x_i = wpool.tile([128, fd], I32, tag="x")
                nc.sync.dma_start(out=x_i[:, :], in_=seq_t[n])

                w = mpool.tile([128, fd], F32, tag="scr", bufs=12)
                nc.scalar.activation(w[:, :], x_i[:, :], SIGN)

                thi = mpool.tile([128, fd], F32, tag="scr", bufs=12)
                nc.vector.tensor_tensor(thi[:, :], x_i[:, :], ahi_sb[:, :], AOT.mult)
                tlo = mpool.tile([128, fd], F32, tag="scr", bufs=12)
                nc.vector.tensor_tensor(tlo[:, :], x_i[:, :], alo_sb[:, :], AOT.mult)

                # continuous prefix sums over the whole 2048-wide tile (exact:
                # chi <= 32*64*7*1023 + 12 < 2^24, clo <= 32*64*7*1023 + 57 < 2^24)
                chi = mpool.tile([128, fd], F32, tag="scr", bufs=12)
                nc.vector.tensor_tensor_scan(
                    chi[:, :], thi[:, :], thi[:, :], b_hi, AOT.add, AOT.bypass
                )
                clo = mpool.tile([128, fd], F32, tag="scr", bufs=12)
                nc.vector.tensor_tensor_scan(
                    clo[:, :], tlo[:, :], tlo[:, :], b_lo, AOT.add, AOT.bypass
                )
                chi3 = chi[:, :].rearrange("p (r l) -> p r l", l=L)
                clo3 = clo[:, :].rearrange("p (r l) -> p r l", l=L)

                nc.scalar.activation(
                    Bhi3[:, 1:rb, :], chi3[:, 0 : rb - 1, L - 1 : L], COPY, bias=-b_hi
                )
                nc.scalar.activation(
                    Blo3[:, 1:rb, :], clo3[:, 0 : rb - 1, L - 1 : L], COPY, bias=-b_lo
                )

                shi = mpool.tile([128, fd], F32, tag="shi")
                shi3 = shi[:, :].rearrange("p (r l) -> p r l", l=L)
                nc.vector.tensor_tensor(
                    shi3, chi3, Bhi3.broadcast_to([128, rb, L]), AOT.subtract
                )
                slo = mpool.tile([128, fd], F32, tag="slo")
                slo3 = slo[:, :].rearrange("p (r l) -> p r l", l=L)
                nc.vector.tensor_tensor(
                    slo3, clo3, Blo3.broadcast_to([128, rb, L]), AOT.subtract
                )

                accf = mpool.tile([128, fd], F32, tag="scr", bufs=12)
                nc.vector.scalar_tensor_tensor(
                    accf[:, :], shi[:, :], float(SCALE), slo[:, :], AOT.mult, AOT.add
                )
                t = mpool.tile([128, fd], F32, tag="scr", bufs=12)
                nc.scalar.activation(t[:, :], accf[:, :], COPY, bias=-500001.0)
                q0 = mpool.tile([128, fd], I32, tag="q0")
                nc.scalar.activation(q0[:, :], t[:, :], COPY, scale=C1)
                # qhp first: it gates the V chain (s1); qp5/qm5 are needed later
                qhp = mpool.tile([128, fd], F32, tag="scr", bufs=12)
                nc.scalar.activation(qhp[:, :], q0[:, :], COPY, scale=P_HI_SC)

                # V fills the ACT round-trip with the ragged-tail work
                lens = mpool.tile([128, rb, 1], F32, tag="lens")
                nc.vector.tensor_reduce(
                    lens[:, :, :],
                    w[:, :].rearrange("p (r l) -> p r l", l=L),
                    mybir.AxisListType.X,
                    AOT.add,
                )
                nmask = mpool.tile([128, fd], mybir.dt.uint32, tag="nmask")
                nmask3 = nmask[:, :].rearrange("p (r l) -> p r l", l=L)
                nc.vector.tensor_tensor(
                    nmask3, io3, lens[:, :, :].broadcast_to([128, rb, L]), AOT.is_ge
                )
                lm1 = mpool.tile([128, rb, 1], F32, tag="lm1")
                nc.scalar.activation(
                    lm1[:, :, :], lens[:, :, :], RELU, bias=neg1[:, :]
                )
                qp5 = mpool.tile([128, fd], F32, tag="scr", bufs=12)
                nc.scalar.activation(qp5[:, :], q0[:, :], COPY, bias=0.5)
                qm5 = mpool.tile([128, fd], F32, tag="scr", bufs=12)
                nc.scalar.activation(qm5[:, :], q0[:, :], COPY, bias=-0.5)

                s1 = mpool.tile([128, fd], F32, tag="scr", bufs=12)
                nc.vector.tensor_tensor(s1[:, :], t[:, :], qhp[:, :], AOT.subtract)
                rxd = mpool.tile([128, fd], F32, tag="scr", bufs=12)
                nc.vector.scalar_tensor_tensor(
                    rxd[:, :], q0[:, :], -float(P_LO), s1[:, :], AOT.mult, AOT.add
                )
                oh = mpool.tile([128, fd], I32, tag="oh")
                oh3 = oh[:, :].rearrange("p (r l) -> p r l", l=L)
                nc.vector.tensor_tensor(
                    oh3, io3.bitcast(I32),
                    lm1[:, :, :].bitcast(I32).broadcast_to([128, rb, L]), AOT.is_equal
                )
                G = mpool.tile([128, fd], F32, tag="scr", bufs=12)
                nc.scalar.activation(
                    G[:, :], rxd[:, :], COPY, scale=-2.0, bias=float(PRIME)
                )
                Gm = mpool.tile([128, fd], F32, tag="scr", bufs=12)
                nc.scalar.activation(
                    Gm[:, :], rxd[:, :], COPY, scale=-2.0, bias=-float(PRIME)
                )
                ebu = mpool.tile([128, fd], I32, tag="scr", bufs=12)
                nc.vector.tensor_scalar(
                    ebu[:, :], qp5[:, :].bitcast(I32), 0x7F800000, None, AOT.bitwise_and
                )
                ebd = mpool.tile([128, fd], I32, tag="scr", bufs=12)
                nc.vector.tensor_scalar(
                    ebd[:, :], qm5[:, :].bitcast(I32), 0x7F800000, None, AOT.bitwise_and
                )
                Vu = mpool.tile([128, fd], F32, tag="scr", bufs=12)
                nc.scalar.activation(Vu[:, :], ebu[:, :].bitcast(F32), COPY, scale=C3)
                Vd = mpool.tile([128, fd], F32, tag="scr", bufs=12)
                nc.scalar.activation(Vd[:, :], ebd[:, :].bitcast(F32), COPY, scale=C3)
                up = mpool.tile([128, fd], I32, tag="scr", bufs=12)
                nc.vector.tensor_tensor(
                    up[:, :], Vu[:, :].bitcast(I32), G[:, :].bitcast(I32), AOT.is_ge
                )
                down = mpool.tile([128, fd], I32, tag="scr", bufs=12)
                nc.vector.tensor_tensor(
                    down[:, :], Gm[:, :].bitcast(I32), Vd[:, :].bitcast(I32), AOT.is_ge
                )
                du = mpool.tile([128, fd], I32, tag="scr", bufs=12)
                nc.vector.tensor_tensor(du[:, :], up[:, :], down[:, :], AOT.subtract)

                u2 = mpool.tile([128, fd], F32, tag="scr", bufs=12)
                nc.vector.scalar_tensor_tensor(
                    u2[:, :], q0[:, :], -float(P_HI), shi[:, :], AOT.mult, AOT.add
                )
                v2 = mpool.tile([128, fd], F32, tag="scr", bufs=12)
                nc.vector.scalar_tensor_tensor(
                    v2[:, :], q0[:, :], -float(P_LO), slo[:, :], AOT.mult, AOT.add
                )
                r0 = mpool.tile([128, fd], F32, tag="scr", bufs=12)
                nc.vector.scalar_tensor_tensor(
                    r0[:, :], u2[:, :], float(SCALE), v2[:, :], AOT.mult, AOT.add
                )
                rref = mpool.tile([128, fd], I32, tag="scr", bufs=12)
                nc.vector.scalar_tensor_tensor(
                    rref[:, :], du[:, :], -float(PRIME), r0[:, :], AOT.mult, AOT.add
                )
                pid = mpool.tile([128, fd], I32, tag="scr", bufs=12)
                nc.vector.tensor_scalar(
                    pid[:, :], rref[:, :], 65535, None, AOT.bitwise_and
                )
                pidf = mpool.tile([128, fd], F32, tag="pidf")
                nc.scalar.activation(pidf[:, :], pid[:, :], COPY)
                pidf3 = pidf[:, :].rearrange("p (r l) -> p r l", l=L)

                ohp = mpool.tile([128, fd], F32, tag="scr", bufs=12)
                nc.vector.tensor_tensor(ohp[:, :], oh[:, :], pidf[:, :], AOT.mult)
                C = mpool.tile([128, rb, 1], F32, tag="C")
                nc.vector.tensor_reduce(
                    C[:, :, :],
                    ohp[:, :].rearrange("p (r l) -> p r l", l=L),
                    mybir.AxisListType.X,
                    AOT.add,
                )
                C3b = C[:, :, :].broadcast_to([128, rb, L])

                nc.vector.copy_predicated(pidf3, nmask3, C3b)

                o = wpool.tile([128, fd], I32, tag="o")
                nc.scalar.activation(o[:, :], pidf[:, :], COPY)

                nc.sync.dma_start(out=out_t[n], in_=o[:, :])

    return nc


_NC_CACHE: dict = {}


def _get_nc(b_val: int):
    key = (int(b_val), ROWS_PER_CORE, FD)
    if key not in _NC_CACHE:
        _NC_CACHE[key] = build_nc(int(b_val))
    return _NC_CACHE[key]


def make_const_inputs(a: np.ndarray, fd: int = FD):
    rb = fd // L
    a64 = a.astype(np.int64)
    ahi_rep = np.tile((a64 >> SHIFT).astype(np.float32), (128, rb))
    alo_rep = np.tile((a64 & (SCALE - 1)).astype(np.float32), (128, rb))
    io0_rep = np.tile(np.arange(L, dtype=np.float32), (128, rb))
    return ahi_rep, alo_rep, io0_rep


def make_in_maps(sequences: np.ndarray, a: np.ndarray):
    ahi_rep, alo_rep, io0_rep = make_const_inputs(a)
    in_maps = []
    for i in range(N_CORES):
        shard = np.ascontiguousarray(
            sequences[i * ROWS_PER_CORE : (i + 1) * ROWS_PER_CORE].astype(
                np.int32, copy=False
            )
        )
        in_maps.append(
            {
                "sequences": shard,
                "ahi_rep": ahi_rep,
                "alo_rep": alo_rep,
                "io0_rep": io0_rep,
            }
        )
    return in_maps


def kernel(sequences: np.ndarray, a: np.ndarray, b) -> np.ndarray:
    sequences = np.asarray(sequences)
    a = np.asarray(a)
    assert sequences.shape == (B_TOTAL, L), sequences.shape

    nc = _get_nc(int(b))
    in_maps = make_in_maps(sequences, a)
    res = run_bass_kernel_spmd(nc, in_maps, core_ids=list(range(N_CORES)))
    outs = [res.results[i]["out"] for i in range(N_CORES)]
    return np.concatenate(outs, axis=0).astype(np.int32, copy=False)


if __name__ == "__main__":
    rng = np.random.default_rng(0)
    seqs = rng.integers(0, 8, size=(B_TOTAL, L), dtype=np.int32)
    a = rng.integers(1, PRIME, size=(L,), dtype=np.int32)
    out = kernel(sequences=seqs, a=a, b=12345)
    print(out.shape, out.dtype, out[:2, :8])


# revision 6
# speedup vs baseline: 1.1093x; 1.0024x over previous
"""Trainium2 Bass kernel for nn_BaseHashCode (prefix-hash of ragged sequences).

Reference computation (per row of `sequences` [B, 64], int32 digits 0..7):
    A_t  = b + sum_{i<=t} a_i * x_i                    (int, < 2^29)
    pid_t = (A_t % 1000003) % 65536   with jax-CPU int32 `%` semantics:
            accf = RNE_f32(A); t = accf - 500001.0 (f32); d = t / p (f32);
            q = round_half_away(d); r = A - q*p (int32); pid = r & 0xffff
    len   = #nonzero digits in the row
    out_t = pid_t if t < len else pid_{len-1}   (len==0 -> pid_0; all equal)

Pure data parallel over 8 NeuronCores (batch shard).  Per core, batch-major
tiles [128 partitions x 1024 free] (16 rows of 64 per partition).

Engine split (v2): the scalar/ACT engine takes every affine/convert unary op,
gpsimd only the two exponent-mask bitwise ANDs, and the vector engine the
tensor-tensor/scan/reduce work.  The modulus is exact in fp32 pieces via a
12/12 split of a (ahi = a>>11 < 512, alo = a&0x7ff < 2048), which keeps a
single continuous 1024-long prefix scan exact (< 2^24) per tile; per-64-block
carries are subtracted afterwards.  q is corrected to the oracle's
round-half-away(f32-division) with ulp-exact thresholds (see derivation in
comments below).
"""

import json

import numpy as np

import concourse.bass as bass
import concourse.mybir as mybir
from concourse.tile import TileContext
from concourse.bass_utils import run_bass_kernel_spmd


# ---------------------------------------------------------------------------
# BIR fixup: this container's walrus rejects instructions with too many
# sync_info.on_wait entries ("Too many sync wait commands").  Hoist excess
# waits onto injected same-engine NoOp instructions placed just before the
# offending instruction (same engine stream => identical semantics).  Only
# monotone waits (sem-ge-imm) are hoisted; eq-style waits stay put.
# ---------------------------------------------------------------------------
_WAIT_LIMIT = 1


def _fix_bir_sync_waits(bir_bytes: bytes, limit: int = _WAIT_LIMIT) -> bytes:
    bir = json.loads(bir_bytes)
    n_fixed = [0]

    def fix_list(insts):
        out = []
        for inst in insts:
            si = inst.get("sync_info") or {}
            ow = si.get("on_wait") or []
            if len(ow) > limit:
                movable = [w for w in ow if w.get("wait_mode") == "sem-ge-imm"]
                fixed = [w for w in ow if w.get("wait_mode") != "sem-ge-imm"]
                keep = (fixed + movable)[:limit]
                hoist = (fixed + movable)[limit:]
                if any(w.get("wait_mode") != "sem-ge-imm" for w in hoist):
                    out.append(inst)
                    continue
                for k in range(0, len(hoist), limit):
                    chunk = hoist[k : k + limit]
                    n_fixed[0] += 1
                    out.append(
                        {
                            "debug": inst.get("debug", 0),
                            "engine": inst["engine"],
                            "ins": [],
                            "name": f"{inst['name']}-wf{k}",
                            "opcode": "NoOp",
                            "outs": [],
                            "sync_info": {"on_wait": chunk},
                        }
                    )
                si = dict(si)
                si["on_wait"] = keep
                inst = dict(inst)
                inst["sync_info"] = si
            out.append(inst)
        return out

    def walk(o):
        if isinstance(o, dict):
            for k, v in o.items():
                if k == "instructions" and isinstance(v, list):
                    o[k] = fix_list(v)
                else:
                    walk(v)
        elif isinstance(o, list):
            for v in o:
                walk(v)

    walk(bir)
    if n_fixed[0]:
        return json.dumps(bir).encode()
    return bir_bytes


def _install_compile_patch():
    import concourse.bass_utils as bu
    import concourse.bass2jax as b2j

    if getattr(bu.compile_bir_kernel, "_waitfix", False):
        return
    orig = bu.compile_bir_kernel

    def patched(bir_json, tmpdir, neff_name="file.neff"):
        return orig(_fix_bir_sync_waits(bir_json), tmpdir, neff_name=neff_name)

    patched._waitfix = True
    bu.compile_bir_kernel = patched
    b2j.compile_bir_kernel = patched


_install_compile_patch()


PRIME = 1_000_003
SHIFT = 10
SCALE = 1 << SHIFT            # 1024
P_HI = PRIME >> SHIFT         # 976
P_LO = PRIME & (SCALE - 1)    # 579   (976*1024 + 579 == 1000003)
P_HI_SC = float(P_HI * SCALE) # 999424.0
L = 64
N_CORES = 8
B_TOTAL = 1_048_576
ROWS_PER_CORE = B_TOTAL // N_CORES  # 131072

FD = 2048                    # free-dim elements per tile
RB = FD // L                 # rows per partition per tile
TILE_ROWS = 128 * RB
N_TILES = ROWS_PER_CORE // TILE_ROWS

AOT = mybir.AluOpType
F32 = mybir.dt.float32
I32 = mybir.dt.int32
COPY = mybir.ActivationFunctionType.Copy
SIGN = mybir.ActivationFunctionType.Sign
RELU = mybir.ActivationFunctionType.Relu

C1 = float(np.float32(1.0) / np.float32(PRIME))
C3 = float(np.float32(PRIME / (1 << 23)))


def build_nc(b_val: int, rows: int = ROWS_PER_CORE, fd: int = FD):
    rb = fd // L
    tile_rows = 128 * rb
    n_tiles = rows // tile_rows
    assert rows % tile_rows == 0
    b_hi = float(int(b_val) >> SHIFT)
    b_lo = float(int(b_val) & (SCALE - 1))

    nc = bass.Bass(target_bir_lowering=False)
    seq = nc.declare_dram_parameter("sequences", [rows, L], I32, isOutput=False)
    ahi_rep = nc.declare_dram_parameter("ahi_rep", [128, fd], F32, isOutput=False)
    alo_rep = nc.declare_dram_parameter("alo_rep", [128, fd], F32, isOutput=False)
    io0_rep = nc.declare_dram_parameter("io0_rep", [128, fd], F32, isOutput=False)
    out = nc.declare_dram_parameter("out", [rows, L], I32, isOutput=True)

    seq_t = seq.rearrange("(n p r) l -> n p (r l)", p=128, r=rb)
    out_t = out.rearrange("(n p r) l -> n p (r l)", p=128, r=rb)

    with TileContext(nc) as tc:
        with (
            tc.tile_pool(name="consts", bufs=1) as cpool,
            tc.tile_pool(name="work", bufs=2) as wpool,
            tc.tile_pool(name="mid", bufs=1) as mpool,
        ):
            ahi_sb = cpool.tile([128, fd], F32, tag="ahi")
            alo_sb = cpool.tile([128, fd], F32, tag="alo")
            io_sb = cpool.tile([128, fd], F32, tag="io")
            nc.sync.dma_start(out=ahi_sb[:, :], in_=ahi_rep[:, :])
            nc.sync.dma_start(out=alo_sb[:, :], in_=alo_rep[:, :])
            nc.sync.dma_start(out=io_sb[:, :], in_=io0_rep[:, :])
            io3 = io_sb[:, :].rearrange("p (r l) -> p r l", l=L)

            neg1 = cpool.tile([128, 1], F32, tag="neg1")
            nc.gpsimd.memset(neg1[:, :], -1.0)
            # per-64-block carry tiles; column 0 is the zero carry of block 0
            Bhi = cpool.tile([128, rb], F32, tag="Bhi")
            Blo = cpool.tile([128, rb], F32, tag="Blo")
            nc.gpsimd.memset(Bhi[:, 0:1], 0.0)
            nc.gpsimd.memset(Blo[:, 0:1], 0.0)
            Bhi3 = Bhi[:, :].rearrange("p (r o) -> p r o", o=1)
            Blo3 = Blo[:, :].rearrange("p (r o) -> p r o", o=1)

            for n in range(n_tiles):
                x_i = wpool.tile([128, fd], I32, tag="x")
                nc.sync.dma_start(out=x_i[:, :], in_=seq_t[n])

                w = mpool.tile([128, fd], F32, tag="scr", bufs=12)
                nc.scalar.activation(w[:, :], x_i[:, :], SIGN)

                thi = mpool.tile([128, fd], F32, tag="scr", bufs=12)
                nc.vector.tensor_tensor(thi[:, :], x_i[:, :], ahi_sb[:, :], AOT.mult)
                tlo = mpool.tile([128, fd], F32, tag="scr", bufs=12)
                nc.vector.tensor_tensor(tlo[:, :], x_i[:, :], alo_sb[:, :], AOT.mult)

                # continuous prefix sums over the whole 2048-wide tile (exact:
                # chi <= 32*64*7*1023 + 12 < 2^24, clo <= 32*64*7*1023 + 57 < 2^24)
                chi = mpool.tile([128, fd], F32, tag="scr", bufs=12)
                nc.vector.tensor_tensor_scan(
                    chi[:, :], thi[:, :], thi[:, :], b_hi, AOT.add, AOT.bypass
                )
                clo = mpool.tile([128, fd], F32, tag="scr", bufs=12)
                nc.vector.tensor_tensor_scan(
                    clo[:, :], tlo[:, :], tlo[:, :], b_lo, AOT.add, AOT.bypass
                )
                chi3 = chi[:, :].rearrange("p (r l) -> p r l", l=L)
                clo3 = clo[:, :].rearrange("p (r l) -> p r l", l=L)

                nc.scalar.activation(
                    Bhi3[:, 1:rb, :], chi3[:, 0 : rb - 1, L - 1 : L], COPY, bias=-b_hi
                )
                nc.scalar.activation(
                    Blo3[:, 1:rb, :], clo3[:, 0 : rb - 1, L - 1 : L], COPY, bias=-b_lo
                )

                shi = mpool.tile([128, fd], F32, tag="shi")
                shi3 = shi[:, :].rearrange("p (r l) -> p r l", l=L)
                nc.vector.tensor_tensor(
                    shi3, chi3, Bhi3.broadcast_to([128, rb, L]), AOT.subtract
                )
                slo = mpool.tile([128, fd], F32, tag="slo")
                slo3 = slo[:, :].rearrange("p (r l) -> p r l", l=L)
                nc.vector.tensor_tensor(
                    slo3, clo3, Blo3.broadcast_to([128, rb, L]), AOT.subtract
                )

                accf = mpool.tile([128, fd], F32, tag="scr", bufs=12)
                nc.vector.scalar_tensor_tensor(
                    accf[:, :], shi[:, :], float(SCALE), slo[:, :], AOT.mult, AOT.add
                )
                t = mpool.tile([128, fd], F32, tag="scr", bufs=12)
                nc.scalar.activation(t[:, :], accf[:, :], COPY, bias=-500001.0)

                # V covers the ACT hop with the ragged-tail work, then computes
                # q0 itself (one less serial ACT hop before s1)
                lens = mpool.tile([128, rb, 1], F32, tag="lens")
                nc.vector.tensor_reduce(
                    lens[:, :, :],
                    w[:, :].rearrange("p (r l) -> p r l", l=L),
                    mybir.AxisListType.X,
                    AOT.add,
                )
                nmask = mpool.tile([128, fd], mybir.dt.uint32, tag="nmask")
                nmask3 = nmask[:, :].rearrange("p (r l) -> p r l", l=L)
                nc.vector.tensor_tensor(
                    nmask3, io3, lens[:, :, :].broadcast_to([128, rb, L]), AOT.is_ge
                )
                q0 = mpool.tile([128, fd], I32, tag="q0")
                nc.vector.tensor_scalar(q0[:, :], t[:, :], C1, None, AOT.mult)

                qhp = mpool.tile([128, fd], F32, tag="scr", bufs=12)
                nc.scalar.activation(qhp[:, :], q0[:, :], COPY, scale=P_HI_SC)
                qp5 = mpool.tile([128, fd], F32, tag="scr", bufs=12)
                nc.scalar.activation(qp5[:, :], q0[:, :], COPY, bias=0.5)
                qm5 = mpool.tile([128, fd], F32, tag="scr", bufs=12)
                nc.scalar.activation(qm5[:, :], q0[:, :], COPY, bias=-0.5)
                lm1 = mpool.tile([128, rb, 1], F32, tag="lm1")
                nc.scalar.activation(
                    lm1[:, :, :], lens[:, :, :], RELU, bias=neg1[:, :]
                )

                oh = mpool.tile([128, fd], I32, tag="oh")
                oh3 = oh[:, :].rearrange("p (r l) -> p r l", l=L)
                nc.vector.tensor_tensor(
                    oh3, io3.bitcast(I32),
                    lm1[:, :, :].bitcast(I32).broadcast_to([128, rb, L]), AOT.is_equal
                )
                s1 = mpool.tile([128, fd], F32, tag="scr", bufs=12)
                nc.vector.tensor_tensor(s1[:, :], t[:, :], qhp[:, :], AOT.subtract)
                rxd = mpool.tile([128, fd], F32, tag="scr", bufs=12)
                nc.vector.scalar_tensor_tensor(
                    rxd[:, :], q0[:, :], -float(P_LO), s1[:, :], AOT.mult, AOT.add
                )
                G = mpool.tile([128, fd], F32, tag="scr", bufs=12)
                nc.scalar.activation(
                    G[:, :], rxd[:, :], COPY, scale=-2.0, bias=float(PRIME)
                )
                Gm = mpool.tile([128, fd], F32, tag="scr", bufs=12)
                nc.scalar.activation(
                    Gm[:, :], rxd[:, :], COPY, scale=-2.0, bias=-float(PRIME)
                )
                ebu = mpool.tile([128, fd], I32, tag="scr", bufs=12)
                nc.vector.tensor_scalar(
                    ebu[:, :], qp5[:, :].bitcast(I32), 0x7F800000, None, AOT.bitwise_and
                )
                ebd = mpool.tile([128, fd], I32, tag="scr", bufs=12)
                nc.vector.tensor_scalar(
                    ebd[:, :], qm5[:, :].bitcast(I32), 0x7F800000, None, AOT.bitwise_and
                )
                Vu = mpool.tile([128, fd], F32, tag="scr", bufs=12)
                nc.scalar.activation(Vu[:, :], ebu[:, :].bitcast(F32), COPY, scale=C3)
                Vd = mpool.tile([128, fd], F32, tag="scr", bufs=12)
                nc.scalar.activation(Vd[:, :], ebd[:, :].bitcast(F32), COPY, scale=C3)

                # u2/v2 are independent of Vu/Vd: they fill the ACT round-trip
                u2 = mpool.tile([128, fd], F32, tag="scr", bufs=12)
                nc.vector.scalar_tensor_tensor(
                    u2[:, :], q0[:, :], -float(P_HI), shi[:, :], AOT.mult, AOT.add
                )
                v2 = mpool.tile([128, fd], F32, tag="scr", bufs=12)
                nc.vector.scalar_tensor_tensor(
                    v2[:, :], q0[:, :], -float(P_LO), slo[:, :], AOT.mult, AOT.add
                )
                up = mpool.tile([128, fd], I32, tag="scr", bufs=12)
                nc.vector.tensor_tensor(
                    up[:, :], Vu[:, :].bitcast(I32), G[:, :].bitcast(I32), AOT.is_ge
                )
                down = mpool.tile([128, fd], I32, tag="scr", bufs=12)
                nc.vector.tensor_tensor(
                    down[:, :], Gm[:, :].bitcast(I32), Vd[:, :].bitcast(I32), AOT.is_ge
                )
                du = mpool.tile([128, fd], I32, tag="scr", bufs=12)
                nc.vector.tensor_tensor(du[:, :], up[:, :], down[:, :], AOT.subtract)

                r0 = mpool.tile([128, fd], F32, tag="scr", bufs=12)
                nc.vector.scalar_tensor_tensor(
                    r0[:, :], u2[:, :], float(SCALE), v2[:, :], AOT.mult, AOT.add
                )
                rref = mpool.tile([128, fd], I32, tag="scr", bufs=12)
                nc.vector.scalar_tensor_tensor(
                    rref[:, :], du[:, :], -float(PRIME), r0[:, :], AOT.mult, AOT.add
                )
                pid = mpool.tile([128, fd], I32, tag="scr", bufs=12)
                nc.vector.tensor_scalar(
                    pid[:, :], rref[:, :], 65535, None, AOT.bitwise_and
                )
                pidf = mpool.tile([128, fd], F32, tag="pidf")
                nc.scalar.activation(pidf[:, :], pid[:, :], COPY)
                pidf3 = pidf[:, :].rearrange("p (r l) -> p r l", l=L)

                ohp = mpool.tile([128, fd], F32, tag="scr", bufs=12)
                nc.vector.tensor_tensor(ohp[:, :], oh[:, :], pidf[:, :], AOT.mult)
                C = mpool.tile([128, rb, 1], F32, tag="C")
                nc.vector.tensor_reduce(
                    C[:, :, :],
                    ohp[:, :].rearrange("p (r l) -> p r l", l=L),
                    mybir.AxisListType.X,
                    AOT.add,
                )
                C3b = C[:, :, :].broadcast_to([128, rb, L])

                nc.vector.copy_predicated(pidf3, nmask3, C3b)

                o = wpool.tile([128, fd], I32, tag="o")
                nc.scalar.activation(o[:, :], pidf[:, :], COPY)

                nc.sync.dma_start(out=out_t[n], in_=o[:, :])

    return nc


_NC_CACHE: dict = {}


def _get_nc(b_val: int):
    key = (int(b_val), ROWS_PER_CORE, FD)
    if key not in _NC_CACHE:
        _NC_CACHE[key] = build_nc(int(b_val))
    return _NC_CACHE[key]


def make_const_inputs(a: np.ndarray, fd: int = FD):
    rb = fd // L
    a64 = a.astype(np.int64)
    ahi_rep = np.tile((a64 >> SHIFT).astype(np.float32), (128, rb))
    alo_rep = np.tile((a64 & (SCALE - 1)).astype(np.float32), (128, rb))
    io0_rep = np.tile(np.arange(L, dtype=np.float32), (128, rb))
    return ahi_rep, alo_rep, io0_rep


def make_in_maps(sequences: np.ndarray, a: np.ndarray):
    ahi_rep, alo_rep, io0_rep = make_const_inputs(a)
    in_maps = []
    for i in range(N_CORES):
        shard = np.ascontiguousarray(
            sequences[i * ROWS_PER_CORE : (i + 1) * ROWS_PER_CORE].astype(
                np.int32, copy=False
            )
        )
        in_maps.append(
            {
                "sequences": shard,
                "ahi_rep": ahi_rep,
                "alo_rep": alo_rep,
                "io0_rep": io0_rep,
            }
        )
    return in_maps


def kernel(sequences: np.ndarray, a: np.ndarray, b) -> np.ndarray:
    sequences = np.asarray(sequences)
    a = np.asarray(a)
    assert sequences.shape == (B_TOTAL, L), sequences.shape

    nc = _get_nc(int(b))
    in_maps = make_in_maps(sequences, a)
    res = run_bass_kernel_spmd(nc, in_maps, core_ids=list(range(N_CORES)))
    outs = [res.results[i]["out"] for i in range(N_CORES)]
    return np.concatenate(outs, axis=0).astype(np.int32, copy=False)


if __name__ == "__main__":
    rng = np.random.default_rng(0)
    seqs = rng.integers(0, 8, size=(B_TOTAL, L), dtype=np.int32)
    a = rng.integers(1, PRIME, size=(L,), dtype=np.int32)
    out = kernel(sequences=seqs, a=a, b=12345)
    print(out.shape, out.dtype, out[:2, :8])
